# revision 3
# baseline (speedup 1.0000x reference)
"""Multi-head causal attention (B=2, S=2048, D=1024, H=16, HD=64) on 8 TRN2 cores.

Sharding: core c handles batch b = c//4 and heads 4*(c%4)..4*(c%4)+3.
The reference reshapes [b,h,s,hd] -> [b,s,1024] WITHOUT head transpose-back,
so output rows [128h, 128h+128) of y[b] depend only on head h: each core
produces a disjoint [512, 1024] block of the output. No collectives.

Per-core kernel (all matmuls float32r, N>=256, base partition 0):
  P1 QKV:  Q^T/K^T per head-pair stacked [128, 2048] (head B extracted to a
           base-0 [64, 2048] tile via partition-shifting SBUF->SBUF DMA);
           V packed [128(s), 16 s-tiles x (4 heads x 64 + ones col)].
  P2 attn: per pair, per q-block j (512), per k-tile t (0..4j+3):
           psS2[128,1024] = scores^T both heads; one ACT exp (scale 1/8);
           diagonal tiles masked post-exp by 0/1 mul; PV matmul with
           V|ones (M=65) accumulates attnT + denominator row.
  P3 norm: denom row -> ones-matmul broadcast [64,512] -> reciprocal -> mul.
  P4 proj: y rows = sum_m attnT_norm[:, m::16].T @ Wo[64m:64m+64, :] + bo.
"""

import sys

if "/opt/trn_rl_repo" not in sys.path:
    sys.path.insert(0, "/opt/trn_rl_repo")

from contextlib import ExitStack

import numpy as np

import concourse.bass as bass
import concourse.tile as tile
from concourse import bacc, mybir

F32 = mybir.dt.float32
F32R = mybir.dt.float32r
EXP = mybir.ActivationFunctionType.Exp

B, S, D, H, HD = 2, 2048, 1024, 16, 64
NC = 8
HPC = 4  # heads per core
CT = D // 128  # 8 contraction tiles
QB = 4  # q-blocks of 512
KT = S // 128  # 16 k-tiles
SCALE = 1.0 / 8.0


def build_nc():
    nc = bacc.Bacc("TRN2", target_bir_lowering=False, debug=False)

    xt = nc.dram_tensor("xt", [CT, 128, S], F32R, kind="ExternalInput").ap()
    wq = nc.dram_tensor("wq", [2, CT, 128, 128], F32R, kind="ExternalInput").ap()
    wk = nc.dram_tensor("wk", [2, CT, 128, 128], F32R, kind="ExternalInput").ap()
    wv = nc.dram_tensor("wv", [CT, 128, 256], F32R, kind="ExternalInput").ap()
    wo = nc.dram_tensor("wo", [2, 16, 64, 512], F32R, kind="ExternalInput").ap()
    bo = nc.dram_tensor("bo", [D], F32, kind="ExternalInput").ap()
    masks = nc.dram_tensor("masks", [4, 128, 512], F32, kind="ExternalInput").ap()
    ones_v = nc.dram_tensor("ones_v", [128, KT, 4], F32R, kind="ExternalInput").ap()
    ones_r = nc.dram_tensor("ones_r", [1, 64], F32R, kind="ExternalInput").ap()
    y = nc.dram_tensor("y", [HPC * 128, D], F32, kind="ExternalOutput").ap()

    with tile.TileContext(nc) as tc, ExitStack() as ctx:
        # ---- long-lived sbuf pools
        persist = ctx.enter_context(tc.tile_pool(name="persist", bufs=1))
        qk_pool = ctx.enter_context(tc.tile_pool(name="qk", bufs=1))

        # constants / small inputs
        masks_sb = persist.tile([128, 4, 512], F32, tag="masks")
        nc.sync.dma_start(out=masks_sb[:], in_=masks.rearrange("a p b -> p a b"))
        bo_sb = persist.tile([128, D], F32, tag="bo")
        bo_b = bass.AP(tensor=bo.tensor, offset=bo.offset, ap=[[0, 128], [1, D]])
        nc.sync.dma_start(out=bo_sb[:], in_=bo_b)
        ones_r_sb = persist.tile([1, 64], F32R, tag="ones_r")
        nc.sync.dma_start(out=ones_r_sb[:], in_=ones_r)

        # V: [128(s_local), 16 s-tiles, 4*65] (col 64 of each 65-group = ones)
        v4 = persist.tile([128, KT, 260], F32R, tag="v4")
        nc.sync.dma_start(
            out=v4[:].rearrange("p t (h c) -> p t h c", c=65)[:, :, :, 64:65],
            in_=ones_v.unsqueeze(3),
        )

        # Q^T/K^T stacked per pair + extracted head-B tiles (base partition 0)
        qst = [qk_pool.tile([128, S], F32R, tag=f"qst{p}", name=f"qst{p}") for p in range(2)]
        kst = [qk_pool.tile([128, S], F32R, tag=f"kst{p}", name=f"kst{p}") for p in range(2)]
        qtb = [qk_pool.tile([64, S], F32R, tag=f"qtb{p}", name=f"qtb{p}") for p in range(2)]
        ktb = [qk_pool.tile([64, S], F32R, tag=f"ktb{p}", name=f"ktb{p}") for p in range(2)]

        # ---- P1: QKV projections
        with ExitStack() as p1:
            xt_pool = p1.enter_context(tc.tile_pool(name="xt", bufs=1))
            w_pool = p1.enter_context(tc.tile_pool(name="w", bufs=1))
            ps1 = p1.enter_context(tc.tile_pool(name="ps1", bufs=2, space="PSUM"))

            xt_sb = xt_pool.tile([128, CT, S], F32R, tag="xt")
            for ct in range(CT):
                nc.sync.dma_start(out=xt_sb[:, ct, :], in_=xt[ct])
            wq_sb = w_pool.tile([128, 2, CT, 128], F32R, tag="wq")
            wk_sb = w_pool.tile([128, 2, CT, 128], F32R, tag="wk")
            for p in range(2):
                for ct in range(CT):
                    nc.sync.dma_start(out=wq_sb[:, p, ct, :], in_=wq[p, ct])
                    nc.sync.dma_start(out=wk_sb[:, p, ct, :], in_=wk[p, ct])
            wv_sb = w_pool.tile([128, CT, 256], F32R, tag="wv")
            for ct in range(CT):
                nc.sync.dma_start(out=wv_sb[:, ct, :], in_=wv[ct])

            # Q^T and K^T, one head-pair at a time, M=128 (2 heads stacked)
            for p in range(2):
                for w_sb, dst in ((wq_sb, qst[p]), (wk_sb, kst[p])):
                    for nb in range(QB):
                        ps = ps1.tile([128, 512], F32, tag="psqk")
                        for ct in range(CT):
                            nc.tensor.matmul(
                                ps[:],
                                w_sb[:, p, ct, :],
                                xt_sb[:, ct, bass.ts(nb, 512)],
                                start=(ct == 0),
                                stop=(ct == CT - 1),
                            )
                        nc.vector.tensor_copy(dst[:, bass.ts(nb, 512)], ps[:])
            # V (4 heads packed in N)
            for st in range(KT):
                ps = ps1.tile([128, 256], F32, tag="psv")
                for ct in range(CT):
                    nc.tensor.matmul(
                        ps[:],
                        xt_sb[:, ct, bass.ts(st, 128)],
                        wv_sb[:, ct, :],
                        start=(ct == 0),
                        stop=(ct == CT - 1),
                    )
                nc.vector.tensor_copy(
                    v4[:, st, :].rearrange("p (h c) -> p h c", c=65)[:, :, 0:64],
                    ps[:].rearrange("p (h c) -> p h c", c=64),
                )

        # extract head-B halves to base-partition-0 tiles (partition-shift DMA)
        for p in range(2):
            nc.sync.dma_start(out=qtb[p][:], in_=qst[p][64:128, :])
            nc.sync.dma_start(out=ktb[p][:], in_=kst[p][64:128, :])

        # ---- P2: attention
        att = ctx.enter_context(tc.tile_pool(name="att", bufs=1))
        attnT = [att.tile([64, S], F32R, tag=f"attnT{h}", name=f"attnT{h}") for h in range(HPC)]
        dh = [att.tile([1, S], F32R, tag=f"d{h}", name=f"dh{h}") for h in range(HPC)]
        wo_pool = ctx.enter_context(tc.tile_pool(name="wo", bufs=1))
        wo_sb = wo_pool.tile([64, 16, 512], F32R, tag="wo")
        for m in range(16):
            nc.sync.dma_start(out=wo_sb[:, m, :], in_=wo[0, m])

        with ExitStack() as p2:
            pt_pool = p2.enter_context(tc.tile_pool(name="pt", bufs=2))
            ps2 = p2.enter_context(tc.tile_pool(name="ps2", bufs=2, space="PSUM"))

            for p in range(2):
                qv = [(qst[p][0:64, :], kst[p][0:64, :]), (qtb[p][:], ktb[p][:])]
                for j in range(QB):
                    psa = [ps2.tile([128, 512], F32, tag=f"psa{q}", name=f"psa{q}") for q in range(2)]
                    for t in range(4 * j + 4):
                        pss = ps2.tile([128, 1024], F32, tag="pss")
                        for q in range(2):
                            qt, kt = qv[q]
                            nc.tensor.matmul(
                                pss[:, bass.ts(q, 512)],
                                kt[:, bass.ts(t, 128)],
                                qt[:, bass.ts(j, 512)],
                                start=True,
                                stop=True,
                            )
                        pt2 = pt_pool.tile([128, 1024], F32R, tag="pt2")
                        nc.scalar.activation(pt2[:], pss[:], EXP, scale=SCALE)
                        r = t - 4 * j
                        if r >= 0:
                            for q in range(2):
                                nc.vector.tensor_mul(
                                    pt2[:, bass.ts(q, 512)],
                                    pt2[:, bass.ts(q, 512)],
                                    masks_sb[:, r, :],
                                )
                        for q in range(2):
                            h = 2 * p + q
                            nc.tensor.matmul(
                                psa[q][0:65, :],
                                v4[:, t, bass.ds(65 * h, 65)],
                                pt2[:, bass.ts(q, 512)],
                                start=(t == 0),
                                stop=(t == 4 * j + 3),
                            )
                    for q in range(2):
                        h = 2 * p + q
                        nc.vector.tensor_copy(
                            attnT[h][:, bass.ts(j, 512)], psa[q][0:64, :]
                        )
                        nc.vector.tensor_copy(
                            dh[h][:, bass.ts(j, 512)], psa[q][64:65, :]
                        )

        # ---- P3: normalize + P4: output projection
        with ExitStack() as p3:
            r_pool = p3.enter_context(tc.tile_pool(name="r", bufs=2))
            ps3 = p3.enter_context(tc.tile_pool(name="ps3", bufs=2, space="PSUM"))
            y_pool = p3.enter_context(tc.tile_pool(name="y", bufs=2))

            for h in range(HPC):
                for cb in range(QB):
                    psr = ps3.tile([64, 512], F32, tag="psr")
                    nc.tensor.matmul(
                        psr[:],
                        ones_r_sb[:],
                        dh[h][:, bass.ts(cb, 512)],
                        start=True,
                        stop=True,
                    )
                    rr = r_pool.tile([64, 512], F32, tag="rr")
                    nc.vector.reciprocal(rr[:], psr[:])
                    nc.vector.tensor_mul(
                        attnT[h][:, bass.ts(cb, 512)],
                        attnT[h][:, bass.ts(cb, 512)],
                        rr[:],
                    )

            for nb in range(2):
                if nb == 1:
                    wo_sb = wo_pool.tile([64, 16, 512], F32R, tag="wo")
                    for m in range(16):
                        nc.sync.dma_start(out=wo_sb[:, m, :], in_=wo[1, m])
                for h in range(HPC):
                    psy = ps3.tile([128, 512], F32, tag="psy")
                    a = attnT[h][:].rearrange("p (r m) -> p m r", m=16)
                    for m in range(16):
                        nc.tensor.matmul(
                            psy[:],
                            a[:, m, :],
                            wo_sb[:, m, :],
                            start=(m == 0),
                            stop=(m == 15),
                        )
                    ys = y_pool.tile([128, 512], F32, tag="ys")
                    nc.vector.tensor_add(ys[:], psy[:], bo_sb[:, bass.ts(nb, 512)])
                    nc.sync.dma_start(
                        out=y[bass.ts(h, 128), bass.ts(nb, 512)], in_=ys[:]
                    )

    nc.compile()
    return nc


def make_masks():
    kl = np.arange(128)[:, None]
    ql = np.arange(512)[None, :]
    return np.stack(
        [(128 * r + kl <= ql).astype(np.float32) for r in range(4)]
    )  # [4, 128, 512]


def prep_core_inputs(c, x, Wq, Wk, Wv, Wo, bo):
    b, g = c // 4, c % 4
    heads = [4 * g + i for i in range(HPC)]
    xt = np.ascontiguousarray(x[b].T).reshape(CT, 128, S)

    def pack_pair(W, p):
        h0, h1 = heads[2 * p], heads[2 * p + 1]
        cols = np.concatenate([W[:, 64 * h0 : 64 * h0 + 64], W[:, 64 * h1 : 64 * h1 + 64]], 1)
        return cols.reshape(CT, 128, 128)

    wq = np.stack([pack_pair(Wq, p) for p in range(2)])
    wk = np.stack([pack_pair(Wk, p) for p in range(2)])
    wv = np.concatenate(
        [Wv[:, 64 * h : 64 * h + 64] for h in heads], 1
    ).reshape(CT, 128, 256)
    wo = np.ascontiguousarray(
        Wo.reshape(16, 64, 2, 512).transpose(2, 0, 1, 3)
    )
    return {
        "xt": xt,
        "wq": wq,
        "wk": wk,
        "wv": wv,
        "wo": wo,
        "bo": bo,
        "masks": make_masks(),
        "ones_v": np.ones((128, KT, 4), np.float32),
        "ones_r": np.ones((1, 64), np.float32),
    }


_NC_CACHE = []


def kernel(x, Wq, Wk, Wv, Wo, bo):
    from concourse import bass_utils

    x, Wq, Wk, Wv, Wo, bo = (
        np.asarray(x, np.float32),
        np.asarray(Wq, np.float32),
        np.asarray(Wk, np.float32),
        np.asarray(Wv, np.float32),
        np.asarray(Wo, np.float32),
        np.asarray(bo, np.float32),
    )
    if not _NC_CACHE:
        _NC_CACHE.append(build_nc())
    nc = _NC_CACHE[0]
    in_maps = [prep_core_inputs(c, x, Wq, Wk, Wv, Wo, bo) for c in range(NC)]
    res = bass_utils.run_bass_kernel_spmd(nc, in_maps, core_ids=list(range(NC)))
    out = np.empty((B, S, D), np.float32)
    for c in range(NC):
        b, g = c // 4, c % 4
        out[b, 512 * g : 512 * (g + 1), :] = res.results[c]["y"]
    return out
